# revision 18
# baseline (speedup 1.0000x reference)
"""CNNSummarizer (CNN encoder + 2-layer LSTM decoder + vocab projection) on 8 trn2 cores.

Sharding:
  - encoder: data-parallel over batch (4 batches per core); one AllGather of the
    per-batch encoder contribution to the LSTM-1 input preactivation (32KB).
  - LSTM recurrence: replicated on all 8 cores (small-collective latency makes
    per-step sharding a loss).
  - vocab projection (H -> V GEMM): column-sharded, 4000 vocab per core (padded
    to 4096 so every vocab tile is 512 wide).

Key structure (v2):
  - Phase 1: index DMAs + embedding gathers are issued before the bulk weight
    DMAs; the dec-token gathers are issued BEFORE the AllGather so the Xdec
    GEMM overlaps the collective.  The encoder contribution Xenc and the layer-1
    bias are folded into the per-step xih tensor during phase 1 (one stacked-
    identity matmul per (m, n) chunk), so the decode loop needs a single
    identity-inject per step for layer 1.
  - Phase 2: per-step vocab GEMM.  Each step's h2^T (128, 4x32 K-chunks) is the
    matmul stationary for an 8x column-tiled (tile_position) vocab GEMM covering
    all 4096 padded vocab columns of this core, emitted in the PE stream at the
    position that covers the LSTM cell latency of the NEXT step.  This keeps the
    tensor engine dense (HAM stays un-throttled) and removes the old 4-step
    lagged 128-row vocab units and the big h2T_all buffer.
  - Logits leave the device as bf16 (half the DMA + faster PSUM eviction);
    host upcasts.

Host-side work is limited to input marshalling: dtype casts of index tensors,
weight transposes/permutations, and the final gather/reshape of the output.
"""

import math
from contextlib import ExitStack

import numpy as np

import concourse.bacc as bacc
import concourse.bass as bass
import concourse.mybir as mybir
import concourse.tile as tile
from concourse.masks import make_identity

V, E, H, F = 32000, 256, 512, 256
FS = (3, 4, 5)
B, S, T = 32, 512, 64
NCORES = 8
BL = B // NCORES          # batches per core
VS = V // NCORES          # true vocab shard per core
VSP = 4096                # padded vocab shard (8 tiles of 512)
TT = T - 1                # decode steps actually computed
G4 = 4 * H                # 2048 gates

dt = mybir.dt
F32 = dt.float32
F32R = dt.float32r
BF16 = dt.bfloat16
AF = mybir.ActivationFunctionType
ALU = mybir.AluOpType
AX = mybir.AxisListType


def _r(ap):
    """View an fp32 AP as float32r for full-rate PE matmuls."""
    return ap.bitcast(F32R)


def build(tt=TT, trace_sim=False):
    """Build the per-core program. All 8 cores run the same NEFF; sharding comes
    from per-core input values."""
    R = tt * B                       # rows of the (t, b) decode matrix
    NM = math.ceil(R / 128)          # m-chunks of decode rows
    NCH = NM                         # dec-token gather chunks (128 tokens each)
    RPAD = NM * 128

    nc = bacc.Bacc("TRN2", target_bir_lowering=False, debug=False,
                   num_devices=NCORES)

    def inp(name, shape, dtype=F32):
        return nc.dram_tensor(name, list(shape), dtype, kind="ExternalInput").ap()

    src_idx = inp("src_idx", (128, (BL * S) // 128), dt.int32)
    dec_idx = inp("dec_idx", (128, NCH), dt.int32)
    enc_emb = inp("enc_emb", (V, E))
    dec_emb = inp("dec_emb", (V, E))
    wconv = {k: inp(f"wconv{k}", (128, k * 4 * 128), BF16) for k in FS}
    bconv = inp("bconv", (128, 2 * len(FS)))   # col = fc*3 + k_idx
    fc1T = inp("fc1T", (128, 6 * H), F32R)
    fc1b = inp("fc1b", (1, H), F32R)
    fc2T = inp("fc2T", (128, 4 * H), F32R)
    fc2b = inp("fc2b", (1, H), F32R)
    WdT = inp("WdT", (128, 2 * G4), BF16)
    WeT = inp("WeT", (128, 4 * G4), BF16)
    b1row = inp("b1row", (1, G4), BF16)
    b2pack = inp("b2pack", (128, H), BF16)
    whh1T = inp("whh1T", (128, 4 * G4), BF16)
    wih2T = inp("wih2T", (128, 4 * G4), BF16)
    whh2T = inp("whh2T", (128, 4 * G4), BF16)
    owT = inp("owT", (128, 4 * VSP), BF16)
    ob8 = inp("ob8", (4, 2 * 512), BF16)  # [k, 512g+d] = out_b[2048g+512k+d]
    blockmask_in = inp("blockmask", (4, 128), BF16)

    out_dram = nc.dram_tensor("logits_sh", [R, VSP], BF16,
                              kind="ExternalOutput").ap()

    with tile.TileContext(nc, trace_sim=trace_sim) as tc:
        with ExitStack() as ctx:
            dram = ctx.enter_context(tc.tile_pool(name="dram", bufs=1,
                                                  space="DRAM"))
            xih_dram = dram.tile([tt, 128, H], BF16)
            cc_in = dram.tile([BL, G4], BF16)
            cc_out = dram.tile([B, G4], BF16, addr_space="Shared")

            const = ctx.enter_context(tc.tile_pool(name="const", bufs=1))
            identF = const.tile([128, 128], F32)
            make_identity(nc, identF[:])
            ident = const.tile([128, 128], F32R)
            nc.vector.tensor_copy(ident[:], identF[:])
            ident_bf128 = const.tile([128, 128], BF16)
            nc.vector.tensor_copy(ident_bf128[:], identF[:])
            scrF = const.tile([128, 128], F32)
            nc.vector.memset(scrF[:], 0.0)
            zpad = const.tile([128, 8], BF16)
            nc.vector.tensor_copy(zpad[:], scrF[:, 0:8])
            nc.vector.memset(scrF[0:1, :], 1.0)
            ones = const.tile([1, 128], F32R)
            nc.vector.tensor_copy(ones[:], scrF[0:1, :])
            ident_bf = const.tile([32, 32], BF16)
            nc.vector.tensor_copy(ident_bf[:], identF[0:32, 0:32])
            ones_bf = const.tile([1, 128], BF16)
            nc.vector.tensor_copy(ones_bf[:], scrF[0:1, :])
            # blockmask[k, p] = 1 iff p // 32 == k  (K=4 stationary that maps
            # 4 bias rows onto the 4 32-partition blocks in one matmul)
            blockmask = const.tile([4, 128], BF16)
            nc.sync.dma_start(blockmask[:], blockmask_in)
            # stacked identity (32, 4x32): out row 32*tau+b = in row b
            stack4 = const.tile([32, 128], BF16)
            for j in range(4):
                nc.vector.tensor_copy(stack4[:, 32 * j:32 * j + 32],
                                      identF[0:32, 0:32])

            # phase-2 weight pool entered before phase-1 pools so the
            # pool stack pops cleanly (DMAs are issued mid-phase-1)
            rw = ctx.enter_context(tc.tile_pool(name="rw", bufs=1))

            # =========================================================
            # Phase 1: encoder (my BL batches) + Xdec GEMM (all rows)
            # =========================================================
            with ExitStack() as p1:
                wpool1 = p1.enter_context(tc.tile_pool(name="wpool1", bufs=1))
                gpool = p1.enter_context(tc.tile_pool(name="gpool", bufs=16))
                tpp = p1.enter_context(tc.tile_pool(name="tpp", bufs=2,
                                                    space="PSUM"))
                cps = p1.enter_context(tc.tile_pool(name="cps", bufs=3,
                                                    space="PSUM"))
                fps = p1.enter_context(tc.tile_pool(name="fps", bufs=2,
                                                    space="PSUM"))
                p1e = ExitStack()
                encp = p1e.enter_context(tc.tile_pool(name="encp", bufs=1))

                # ---- index DMAs FIRST so gathers start immediately ----
                idx_s_sb = encp.tile([128, (BL * S) // 128], dt.int32)
                nc.sync.dma_start(idx_s_sb[:], src_idx)
                idx_d_sb = wpool1.tile([128, NCH], dt.int32)
                nc.sync.dma_start(idx_d_sb[:], dec_idx)

                # ---- issue all enc-embedding gathers up front ----
                gts = []
                for b in range(BL):
                    for ch in range(4):
                        gt = gpool.tile([128, E], F32, tag="gath")
                        nc.gpsimd.indirect_dma_start(
                            out=gt[:], out_offset=None, in_=enc_emb,
                            in_offset=bass.IndirectOffsetOnAxis(
                                ap=idx_s_sb[:, 4 * b + ch:4 * b + ch + 1],
                                axis=0))
                        gts.append(gt)

                # ---- weight DMAs (behind the gathers in queue order) ----
                wconv_sb = {}
                for k in FS:
                    wk = encp.tile([128, k * 4 * 128], BF16,
                                   name=f"wconv{k}_sb")
                    nc.sync.dma_start(wk[:], wconv[k])
                    wconv_sb[k] = wk
                bconv_sb = encp.tile([128, 2 * len(FS)], F32)
                nc.sync.dma_start(bconv_sb[:], bconv)
                fc1T_sb = encp.tile([128, 6 * H], F32R)
                nc.sync.dma_start(fc1T_sb[:], fc1T)
                fc2T_sb = encp.tile([128, 4 * H], F32R)
                nc.sync.dma_start(fc2T_sb[:], fc2T)
                fc1b_sb = encp.tile([1, H], F32R)
                nc.sync.dma_start(fc1b_sb[:], fc1b)
                fc2b_sb = encp.tile([1, H], F32R)
                nc.sync.dma_start(fc2b_sb[:], fc2b)
                WeT_sb = encp.tile([128, 4 * G4], BF16)
                nc.sync.dma_start(WeT_sb[:], WeT)
                b1_sb = encp.tile([1, G4], BF16)
                nc.sync.dma_start(b1_sb[:], b1row)
                WdT_sb = wpool1.tile([128, 2 * G4], BF16)
                nc.sync.dma_start(WdT_sb[:], WdT)

                XPAD = BL * (S + 8)
                SEG = S + 8
                xT_sb = encp.tile([128, 2 * XPAD], BF16)        # [ec] blocks
                dembT_sb = wpool1.tile([128, 2 * RPAD], BF16)   # [ec] blocks

                def evict(dst, src, parity):
                    if parity % 2 == 0:
                        nc.vector.tensor_copy(dst, src)
                    else:
                        nc.scalar.copy(dst, src)

                pooled = encp.tile([128, 6 * BL], F32R)

                def conv_batch(b):
                    for ki, k in enumerate(FS):
                        for fc in range(2):
                            ps = cps.tile([128, 512], F32, tag="conv",
                                          space="PSUM")
                            first = True
                            for j in range(k):
                                for ec in range(2):
                                    lhs = wconv_sb[k][
                                        :, (j * 4 + ec * 2 + fc) * 128:
                                        (j * 4 + ec * 2 + fc) * 128 + 128]
                                    rhs = xT_sb[:, ec * XPAD + SEG * b + j:
                                                ec * XPAD + SEG * b + j + 512]
                                    nc.tensor.matmul(
                                        ps[:], lhs, rhs, start=first,
                                        stop=(j == k - 1 and ec == 1))
                                    first = False
                            kc = ki * 2 + fc
                            nc.vector.tensor_reduce(
                                pooled[:, BL * kc + b: BL * kc + b + 1],
                                ps[:, 0:S - k + 1], axis=AX.X, op=ALU.max)

                for b in range(BL):
                    for ch in range(4):
                        gt = gts[4 * b + ch]
                        for ec in range(2):
                            tp = tpp.tile([128, 128], F32, tag="tp",
                                          space="PSUM")
                            nc.tensor.transpose(
                                tp[:], gt[:, 128 * ec:128 * ec + 128],
                                ident[:].bitcast(F32))
                            evict(xT_sb[:, ec * XPAD + SEG * b + 128 * ch:
                                        ec * XPAD + SEG * b + 128 * ch + 128],
                                  tp[:], ch + ec)
                    for ec in range(2):
                        nc.vector.tensor_copy(
                            xT_sb[:, ec * XPAD + SEG * b + S:
                                  ec * XPAD + SEG * (b + 1)], zpad[:])
                    conv_batch(b)

                for ki in range(len(FS)):
                    for fc in range(2):
                        kc = ki * 2 + fc
                        nc.scalar.activation(
                            pooled[:, BL * kc: BL * kc + BL],
                            pooled[:, BL * kc: BL * kc + BL],
                            AF.Relu, bias=bconv_sb[:, fc * 3 + ki: fc * 3 + ki + 1])

                # ---- fc1 -> relu -> fc2 -> Xenc (into cc_in) ----
                ps1 = fps.tile([BL, H], F32, tag="f", space="PSUM")
                for kc in range(6):
                    nc.tensor.matmul(ps1[:], _r(pooled[:, BL * kc: BL * kc + BL]),
                                     _r(fc1T_sb[:, H * kc: H * kc + H]),
                                     start=(kc == 0), stop=False)
                nc.tensor.matmul(ps1[:], _r(ones[0:1, 0:BL]), _r(fc1b_sb[:]),
                                 start=False, stop=True)
                h1e = encp.tile([BL, H], F32)
                nc.scalar.activation(h1e[:], ps1[:], AF.Relu)

                h1eT = encp.tile([128, 4 * BL], F32R)
                for kc in range(4):
                    tp = tpp.tile([128, 128], F32, tag="tp", space="PSUM")
                    nc.tensor.transpose(tp[0:128, 0:BL],
                                        h1e[:, 128 * kc:128 * kc + 128],
                                        ident[0:BL, 0:BL].bitcast(F32))
                    nc.vector.tensor_copy(h1eT[:, BL * kc:BL * kc + BL],
                                          tp[0:128, 0:BL])

                ps2 = fps.tile([BL, H], F32, tag="f", space="PSUM")
                for kc in range(4):
                    nc.tensor.matmul(ps2[:], _r(h1eT[:, BL * kc:BL * kc + BL]),
                                     _r(fc2T_sb[:, H * kc:H * kc + H]),
                                     start=(kc == 0), stop=False)
                nc.tensor.matmul(ps2[:], _r(ones[0:1, 0:BL]), _r(fc2b_sb[:]),
                                 start=False, stop=True)
                enc_sb = encp.tile([BL, H], F32)
                nc.vector.tensor_copy(enc_sb[:], ps2[:])

                encT = encp.tile([128, 4 * BL], BF16)
                for kc in range(4):
                    tp = tpp.tile([128, 128], F32, tag="tp", space="PSUM")
                    nc.tensor.transpose(tp[0:128, 0:BL],
                                        enc_sb[:, 128 * kc:128 * kc + 128],
                                        ident[0:BL, 0:BL].bitcast(F32))
                    nc.vector.tensor_copy(encT[:, BL * kc:BL * kc + BL],
                                          tp[0:128, 0:BL])

                # xe = enc @ We^T + b1  (per-batch rows of the g1 preact)
                xe_sb = encp.tile([BL, G4], BF16)
                for n in range(4):
                    ps = fps.tile([BL, 512], F32, tag="f", space="PSUM")
                    for kc in range(4):
                        nc.tensor.matmul(
                            ps[:], encT[:, BL * kc:BL * kc + BL],
                            WeT_sb[:, kc * G4 + 512 * n:
                                   kc * G4 + 512 * n + 512],
                            start=(kc == 0), stop=False)
                    nc.tensor.matmul(ps[:], ones_bf[0:1, 0:BL],
                                     b1_sb[:, 512 * n:512 * n + 512],
                                     start=False, stop=True)
                    nc.vector.tensor_copy(xe_sb[:, 512 * n:512 * n + 512], ps[:])
                nc.sync.dma_start(cc_in[:], xe_sb[:])
                p1e.close()

                # ---- dec-token gathers BEFORE the collective so the Xdec
                # work overlaps the AllGather latency ----
                dgs = []
                for m in range(NM):
                    gt = gpool.tile([128, E], F32, tag="gath")
                    nc.gpsimd.indirect_dma_start(
                        out=gt[:], out_offset=None, in_=dec_emb,
                        in_offset=bass.IndirectOffsetOnAxis(
                            ap=idx_d_sb[:, m:m + 1], axis=0))
                    dgs.append(gt)

                nc.gpsimd.collective_compute(
                    "AllGather", ALU.bypass,
                    replica_groups=[list(range(NCORES))],
                    ins=[cc_in.opt()], outs=[cc_out.opt()])

                # phase-2 weight DMAs issued here so the 10MB loads overlap
                # the collective + Xdec GEMM instead of stalling decode start
                b2_sb = rw.tile([128, H], BF16)
                nc.sync.dma_start(b2_sb[:], b2pack)
                whh1_sb = rw.tile([128, 4 * G4], BF16)
                nc.sync.dma_start(whh1_sb[:], whh1T)
                wih2_sb = rw.tile([128, 4 * G4], BF16)
                nc.sync.dma_start(wih2_sb[:], wih2T)
                whh2_sb = rw.tile([128, 4 * G4], BF16)
                nc.sync.dma_start(whh2_sb[:], whh2T)
                ob8_sb = rw.tile([4, 2 * 512], BF16)
                nc.sync.dma_start(ob8_sb[:], ob8)

                for m in range(NM):
                    gt = dgs[m]
                    for ec in range(2):
                        tp = tpp.tile([128, 128], F32, tag="tp", space="PSUM")
                        nc.tensor.transpose(
                            tp[:], gt[:, 128 * ec:128 * ec + 128],
                            ident[:].bitcast(F32))
                        evict(dembT_sb[:, ec * RPAD + 128 * m:
                                       ec * RPAD + 128 * m + 128],
                              tp[:], m + ec)

                # full-batch xe rows (b, G4) for the stacked-identity inject
                xe_rows = wpool1.tile([B, G4], BF16)
                nc.sync.dma_start(xe_rows[:], cc_out)

                # ---- Xdec GEMM (+Xenc+b1 fold) -> packed bf16 xih_dram ----
                xdpool = p1.enter_context(tc.tile_pool(name="xdpool", bufs=2))
                for m in range(NM):
                    tm = min(4, tt - 4 * m)
                    Mm = 32 * tm
                    xd_sb = xdpool.tile([128, G4], BF16, tag="xd_sb")
                    for n in range(4):
                        ps = fps.tile([128, 512], F32, tag="f", space="PSUM")
                        for ec in range(2):
                            nc.tensor.matmul(
                                ps[0:Mm, :],
                                dembT_sb[:, ec * RPAD + 128 * m:
                                         ec * RPAD + 128 * m + Mm],
                                WdT_sb[:, ec * G4 + 512 * n:
                                       ec * G4 + 512 * n + 512],
                                start=(ec == 0), stop=False)
                        nc.tensor.matmul(
                            ps[0:Mm, :], stack4[:, 0:Mm],
                            xe_rows[:, 512 * n:512 * n + 512],
                            start=False, stop=True)
                        evict(xd_sb[0:Mm, 512 * n:512 * n + 512],
                              ps[0:Mm, :], m + n)
                    for tau in range(tm):
                        dst = xih_dram[4 * m + tau].rearrange(
                            "(j b) d -> b j d", j=4)
                        nc.sync.dma_start(dst,
                                          xd_sb[32 * tau:32 * tau + 32, :])

            # =========================================================
            # Phase 2: recurrence with packed gate PSUM, col-tiled GEMMs
            # gate-block order [i, f, o, g] on psum partitions [0:32,...]
            # =========================================================
            with ExitStack() as p2:
                vw = p2.enter_context(tc.tile_pool(name="vw", bufs=1))
                owT_sb = vw.tile([128, 4 * VSP], BF16)
                nc.sync.dma_start(owT_sb[:], owT)
                rp = p2.enter_context(tc.tile_pool(name="rp", bufs=2))
                xp = p2.enter_context(tc.tile_pool(name="xp", bufs=3))
                hp = p2.enter_context(tc.tile_pool(name="hp", bufs=2))
                rps = p2.enter_context(tc.tile_pool(name="rps", bufs=2,
                                                    space="PSUM"))
                tps = p2.enter_context(tc.tile_pool(name="tps", bufs=2,
                                                    space="PSUM"))
                vo = p2.enter_context(tc.tile_pool(name="vo", bufs=3))
                vps = p2.enter_context(tc.tile_pool(name="vps", bufs=2,
                                                    space="PSUM"))

                c1 = rp.tile([64, H], BF16, tag="c1")
                nc.vector.memset(c1[32:64, :], 0.0)
                c2 = rp.tile([64, H], BF16, tag="c2")
                nc.vector.memset(c2[32:64, :], 0.0)

                def cell(ps_g, c_prev, tag):
                    """LSTM cell from packed-gate psum (128, H) -> (h, c_new).

                    Gate blocks keep their partition homes: i/f/o from one
                    96-partition sigmoid, tanh(g) lands at [0:32] (cross-
                    partition ACT is legal). Each 2-input DVE op has both
                    inputs on one block's partitions; outputs may land on a
                    different base. c lives at [32:64]. All elementwise math
                    in bf16 for 2x DVE throughput.
                    """
                    sig = rp.tile([96, H], BF16, tag=f"sig{tag}")
                    nc.scalar.activation(sig[:], ps_g[0:96, :], AF.Sigmoid)
                    tg = rp.tile([32, H], BF16, tag=f"tg{tag}")
                    nc.scalar.activation(tg[:], ps_g[96:128, :], AF.Tanh)
                    c_new = rp.tile([64, H], BF16, tag=f"c{tag}")
                    nc.vector.tensor_mul(c_new[32:64, :], sig[32:64, :],
                                         c_prev[32:64, :])
                    m1 = rp.tile([64, H], BF16, tag=f"m1{tag}")
                    nc.vector.tensor_mul(m1[32:64, :], sig[0:32, :], tg[:])
                    nc.vector.tensor_add(c_new[32:64, :], m1[32:64, :],
                                         c_new[32:64, :])
                    th = rp.tile([96, H], BF16, tag=f"th{tag}")
                    nc.scalar.activation(th[64:96, :], c_new[32:64, :], AF.Tanh)
                    h = rp.tile([32, H], BF16, tag=f"h{tag}")
                    nc.vector.tensor_mul(h[:], sig[64:96, :], th[64:96, :])
                    return h, c_new

                def transpose_state(h, dst_sb):
                    """h (32, H) -> h^T kc-chunks packed (128, 4x32) in SBUF."""
                    tp = tps.tile([128, 128], BF16, tag="tps", space="PSUM")
                    for kc in range(4):
                        nc.tensor.transpose(tp[:, 32 * kc:32 * kc + 32],
                                            h[:, 128 * kc:128 * kc + 128],
                                            ident_bf[:])
                    nc.vector.tensor_copy(dst_sb[:], tp[:])

                def gemm_block(ps, stat, w_sb, final):
                    for kc in range(4):
                        for j in range(4):
                            nc.tensor.matmul(
                                ps[32 * j:32 * j + 32, :],
                                stat[:, 32 * kc:32 * kc + 32],
                                w_sb[:, kc * G4 + 512 * j:
                                     kc * G4 + 512 * j + 512],
                                start=False,
                                stop=(final and kc == 3 and j == 3),
                                skip_group_check=True,
                                tile_position=(0, 32 * j))

                def vocab_step(t, h2T):
                    """Full padded-vocab GEMM for decode step t.

                    Stationary = h2T (128, 4x32 K-chunks); 4-way column tiling
                    over n-tiles; two (128, 512) psum groups cover 4096 cols.
                    A K=1 ones x out_b broadcast matmul opens each group: it
                    clears the bank once (so the j-tiles can all run start=False
                    and overlap) and folds the output bias in on the PE.
                    """
                    for g in range(2):
                        ps = vps.tile([128, 512], F32, tag="vps", space="PSUM")
                        # partition block j gets bias row out_b[512*(4g+j):]
                        # via the blockmask; clears the bank in one full-
                        # partition matmul so the j-tiles can overlap.
                        nc.tensor.matmul(
                            ps[:], blockmask[:],
                            ob8_sb[:, 512 * g:512 * g + 512],
                            start=True, stop=False, skip_group_check=True)
                        for kc in range(4):
                            for j in range(4):
                                nc.tensor.matmul(
                                    ps[32 * j:32 * j + 32, :],
                                    h2T[:, 32 * kc:32 * kc + 32],
                                    owT_sb[:, kc * VSP + 512 * (4 * g + j):
                                           kc * VSP + 512 * (4 * g + j) + 512],
                                    start=False,
                                    stop=(kc == 3 and j == 3),
                                    skip_group_check=True,
                                    tile_position=(0, 32 * j))
                        ob = vo.tile([128, 512], BF16, tag="ob")
                        nc.vector.tensor_copy(ob[:], ps[:])
                        dst = out_dram[32 * t:32 * t + 32,
                                       2048 * g:2048 * g + 2048].rearrange(
                            "b (j d) -> j b d", j=4)
                        nc.sync.dma_start(dst, ob[:])

                # ---- software-pipelined decode loop ----
                xih_t = xp.tile([128, H], BF16, tag="xih")
                nc.sync.dma_start(xih_t[:], xih_dram[0])
                xih_n = xp.tile([128, H], BF16, tag="xih")
                nc.sync.dma_start(xih_n[:], xih_dram[1])

                ps_g1 = rps.tile([128, H], F32, tag="g1", space="PSUM")
                nc.tensor.matmul(ps_g1[:], ident_bf128[:], xih_t[:],
                                 start=True, stop=True, skip_group_check=True)
                h1, c1 = cell(ps_g1, c1, "1")

                # g2(0) init: b2 only (h2(-1) = 0)
                ps_g2 = rps.tile([128, H], F32, tag="g2", space="PSUM")
                nc.tensor.matmul(ps_g2[:], ident_bf128[:], b2_sb[:],
                                 start=True, stop=False, skip_group_check=True)

                h2T_prev = None
                for t in range(tt):
                    # A: h1T(t) transpose  [dep: cell1(t)]
                    h1T = hp.tile([128, 128], BF16, tag="h1T")
                    transpose_state(h1, h1T)
                    # B: wih2 @ h1T(t) -> finish ps_g2(t)
                    gemm_block(ps_g2, h1T, wih2_sb, True)
                    # cell2(t) first in the ACT/DVE queues (dep: B only)
                    h2, c2 = cell(ps_g2, c2, "2")
                    # C: start ps_g1(t+1) = xih(t+1) + whh1 @ h1T(t)
                    if t + 1 < tt:
                        ps_g1 = rps.tile([128, H], F32, tag="g1", space="PSUM")
                        nc.tensor.matmul(ps_g1[:], ident_bf128[:], xih_n[:],
                                         start=True, stop=False,
                                         skip_group_check=True)
                        gemm_block(ps_g1, h1T, whh1_sb, True)
                        # cell1(t+1) interleaves with cell2(t) on ACT/DVE
                        h1, c1 = cell(ps_g1, c1, "1")
                        if t + 2 < tt:
                            xih_n = xp.tile([128, H], BF16, tag="xih")
                            nc.sync.dma_start(xih_n[:], xih_dram[t + 2])
                    # G': vocab(t-1) fills the PE while cell2(t) runs
                    if t > 0:
                        vocab_step(t - 1, h2T_prev)
                    # D: h2T(t) transpose (dep: cell2(t), covered by C+G')
                    h2T_cur = hp.tile([128, 128], BF16, tag="h2T")
                    transpose_state(h2, h2T_cur)
                    # E/F: start ps_g2(t+1) = b2 + whh2 @ h2T(t)
                    if t + 1 < tt:
                        ps_g2 = rps.tile([128, H], F32, tag="g2", space="PSUM")
                        nc.tensor.matmul(ps_g2[:], ident_bf128[:], b2_sb[:],
                                         start=True, stop=False,
                                         skip_group_check=True)
                        gemm_block(ps_g2, h2T_cur, whh2_sb, False)
                    h2T_prev = h2T_cur

                vocab_step(tt - 1, h2T_prev)

    nc.compile()
    return nc


# =====================================================================
# Host side
# =====================================================================

def _bf16(a):
    import ml_dtypes
    return np.ascontiguousarray(a.astype(ml_dtypes.bfloat16))


def _chunk(a):
    """(c*128, X) -> (128, c*X): partition-chunked layout for SBUF tiles."""
    c = a.shape[0] // 128
    return np.ascontiguousarray(
        a.reshape(c, 128, -1).transpose(1, 0, 2).reshape(128, -1))


def host_prep(inputs, tt=TT):
    """Build per-core input maps from the full problem inputs."""
    R = tt * B
    NM = math.ceil(R / 128)
    f32 = lambda a: np.ascontiguousarray(np.asarray(a), dtype=np.float32)
    # gate permutation [i, f, o, g]
    perm = np.concatenate([np.arange(0, H), np.arange(H, 2 * H),
                           np.arange(3 * H, 4 * H), np.arange(2 * H, 3 * H)])

    src = np.asarray(inputs["src"])
    trg = np.asarray(inputs["trg"])

    w_ih1 = f32(inputs["w_ih1"])[perm]
    b1 = (f32(inputs["b_ih1"]) + f32(inputs["b_hh1"]))[perm][None, :]
    b2 = (f32(inputs["b_ih2"]) + f32(inputs["b_hh2"]))[perm]
    b2pack = np.ascontiguousarray(
        np.broadcast_to(b2.reshape(4, 1, H), (4, 32, H)).reshape(128, H))

    shared = {
        "enc_emb": f32(inputs["enc_emb"]),
        "dec_emb": f32(inputs["dec_emb"]),
        "bconv": np.ascontiguousarray(
            np.stack([f32(inputs[f"conv_b{k}"]).reshape(2, 128)[fc]
                      for fc in range(2) for k in FS], axis=1)),
        "fc1T": _chunk(f32(inputs["fc1_w"]).T),
        "fc1b": f32(inputs["fc1_b"])[None, :],
        "fc2T": _chunk(f32(inputs["fc2_w"]).T),
        "fc2b": f32(inputs["fc2_b"])[None, :],
        "WdT": _bf16(_chunk(np.ascontiguousarray(w_ih1[:, :E].T))),
        "WeT": _bf16(_chunk(np.ascontiguousarray(w_ih1[:, E:].T))),
        "b1row": _bf16(b1),
        "b2pack": _bf16(b2pack),
        "whh1T": _bf16(_chunk(np.ascontiguousarray(f32(inputs["w_hh1"])[perm].T))),
        "wih2T": _bf16(_chunk(np.ascontiguousarray(f32(inputs["w_ih2"])[perm].T))),
        "whh2T": _bf16(_chunk(np.ascontiguousarray(f32(inputs["w_hh2"])[perm].T))),
    }
    for k in FS:
        A = f32(inputs[f"conv_w{k}"]).transpose(2, 1, 0)   # (k, E, F)
        A = A.reshape(k, 2, 128, 2, 128).transpose(0, 1, 3, 2, 4)
        shared[f"wconv{k}"] = _bf16(_chunk(A.reshape(k * 4 * 128, 128)))

    dtoks = trg[:, :tt].T.reshape(-1).astype(np.int32)
    dtoks = np.concatenate([dtoks, np.zeros(NM * 128 - R, np.int32)])
    dec_idx = np.ascontiguousarray(dtoks.reshape(NM, 128).T)

    owT_full = np.ascontiguousarray(f32(inputs["out_w"]).T)   # (H, V)
    ob_full = f32(inputs["out_b"])

    in_maps = []
    for c in range(NCORES):
        stoks = src[BL * c: BL * (c + 1)].reshape(-1).astype(np.int32)
        m = dict(shared)
        m["src_idx"] = np.ascontiguousarray(stoks.reshape(-1, 128).T)
        m["dec_idx"] = dec_idx
        ow = np.zeros((H, VSP), np.float32)
        ow[:, :VS] = owT_full[:, VS * c: VS * (c + 1)]
        m["owT"] = _bf16(_chunk(ow))
        ob = np.zeros(VSP, np.float32)
        ob[:VS] = ob_full[VS * c: VS * (c + 1)]
        m["ob8"] = _bf16(np.ascontiguousarray(
            ob.reshape(2, 4, 512).transpose(1, 0, 2).reshape(4, 1024)))
        bm = np.zeros((4, 128), np.float32)
        for k in range(4):
            bm[k, 32 * k:32 * k + 32] = 1.0
        m["blockmask"] = _bf16(bm)
        in_maps.append(m)
    return in_maps


def assemble(results, tt=TT):
    """Gather per-core logit shards -> full (B, T, V) output."""
    out = np.zeros((B, T, V), dtype=np.float32)
    for c, res in enumerate(results):
        sh = np.asarray(res["logits_sh"]).astype(np.float32)
        sh = sh.reshape(tt, B, VSP)[:, :, :VS]
        out[:, 1:1 + tt, VS * c: VS * (c + 1)] = sh.transpose(1, 0, 2)
    return out


_CACHE = {}


def kernel(**inputs):
    if "nc" not in _CACHE:
        _CACHE["nc"] = build()
    nc = _CACHE["nc"]
    from concourse.bass_utils import run_bass_kernel_spmd
    in_maps = host_prep(inputs)
    res = run_bass_kernel_spmd(nc, in_maps, core_ids=list(range(NCORES)))
    return assemble(res.results)


# revision 23
# speedup vs baseline: 1.0056x; 1.0056x over previous
"""CNNSummarizer (CNN encoder + 2-layer LSTM decoder + vocab projection) on 8 trn2 cores.

Sharding:
  - encoder: data-parallel over batch (4 batches per core); one AllGather of the
    per-batch encoder contribution to the LSTM-1 input preactivation (32KB).
  - LSTM recurrence: replicated on all 8 cores (small-collective latency makes
    per-step sharding a loss).
  - vocab projection (H -> V GEMM): column-sharded, 4000 vocab per core (padded
    to 4096 so every vocab tile is 512 wide).

Key structure (v2):
  - Phase 1: index DMAs + embedding gathers are issued before the bulk weight
    DMAs; the dec-token gathers are issued BEFORE the AllGather so the Xdec
    GEMM overlaps the collective.  The encoder contribution Xenc and the layer-1
    bias are folded into the per-step xih tensor during phase 1 (one stacked-
    identity matmul per (m, n) chunk), so the decode loop needs a single
    identity-inject per step for layer 1.
  - Phase 2: per-step vocab GEMM.  Each step's h2^T (128, 4x32 K-chunks) is the
    matmul stationary for an 8x column-tiled (tile_position) vocab GEMM covering
    all 4096 padded vocab columns of this core, emitted in the PE stream at the
    position that covers the LSTM cell latency of the NEXT step.  This keeps the
    tensor engine dense (HAM stays un-throttled) and removes the old 4-step
    lagged 128-row vocab units and the big h2T_all buffer.
  - Logits leave the device as bf16 (half the DMA + faster PSUM eviction);
    host upcasts.

Host-side work is limited to input marshalling: dtype casts of index tensors,
weight transposes/permutations, and the final gather/reshape of the output.
"""

import math
from contextlib import ExitStack

import numpy as np

import concourse.bacc as bacc
import concourse.bass as bass
import concourse.mybir as mybir
import concourse.tile as tile
from concourse.masks import make_identity

V, E, H, F = 32000, 256, 512, 256
FS = (3, 4, 5)
B, S, T = 32, 512, 64
NCORES = 8
BL = B // NCORES          # batches per core
VS = V // NCORES          # true vocab shard per core
VSP = 4096                # padded vocab shard (8 tiles of 512)
TT = T - 1                # decode steps actually computed
G4 = 4 * H                # 2048 gates

dt = mybir.dt
F32 = dt.float32
F32R = dt.float32r
BF16 = dt.bfloat16
AF = mybir.ActivationFunctionType
ALU = mybir.AluOpType
AX = mybir.AxisListType


def _r(ap):
    """View an fp32 AP as float32r for full-rate PE matmuls."""
    return ap.bitcast(F32R)


def build(tt=TT, trace_sim=False):
    """Build the per-core program. All 8 cores run the same NEFF; sharding comes
    from per-core input values."""
    R = tt * B                       # rows of the (t, b) decode matrix
    NM = math.ceil(R / 128)          # m-chunks of decode rows
    NCH = NM                         # dec-token gather chunks (128 tokens each)
    RPAD = NM * 128

    nc = bacc.Bacc("TRN2", target_bir_lowering=False, debug=False,
                   num_devices=NCORES)

    def inp(name, shape, dtype=F32):
        return nc.dram_tensor(name, list(shape), dtype, kind="ExternalInput").ap()

    src_idx = inp("src_idx", (128, (BL * S) // 128), dt.int32)
    dec_idx = inp("dec_idx", (128, NCH), dt.int32)
    enc_emb = inp("enc_emb", (V, E))
    dec_emb = inp("dec_emb", (V, E))
    wconv = {k: inp(f"wconv{k}", (128, k * 4 * 128), BF16) for k in FS}
    bconv = inp("bconv", (128, 2 * len(FS)))   # col = fc*3 + k_idx
    fc1T = inp("fc1T", (128, 6 * H), F32R)
    fc1b = inp("fc1b", (1, H), F32R)
    fc2T = inp("fc2T", (128, 4 * H), F32R)
    fc2b = inp("fc2b", (1, H), F32R)
    WdT = inp("WdT", (128, 2 * G4), BF16)
    WeT = inp("WeT", (128, 4 * G4), BF16)
    b1row = inp("b1row", (1, G4), BF16)
    b2pack = inp("b2pack", (128, H), BF16)
    whh1T = inp("whh1T", (128, 4 * G4), BF16)
    wih2T = inp("wih2T", (128, 4 * G4), BF16)
    whh2T = inp("whh2T", (128, 4 * G4), BF16)
    owT = inp("owT", (128, 4 * VSP), BF16)
    ob8 = inp("ob8", (4, 2 * 512), BF16)  # [k, 512g+d] = out_b[2048g+512k+d]
    blockmask_in = inp("blockmask", (4, 128), BF16)

    out_dram = nc.dram_tensor("logits_sh", [R, VSP], BF16,
                              kind="ExternalOutput").ap()

    with tile.TileContext(nc, trace_sim=trace_sim) as tc:
        with ExitStack() as ctx:
            dram = ctx.enter_context(tc.tile_pool(name="dram", bufs=1,
                                                  space="DRAM"))
            xih_dram = dram.tile([tt, 128, H], BF16)
            cc_in = dram.tile([BL, G4], BF16)
            cc_out = dram.tile([B, G4], BF16, addr_space="Shared")

            const = ctx.enter_context(tc.tile_pool(name="const", bufs=1))
            identF = const.tile([128, 128], F32)
            make_identity(nc, identF[:])
            ident = const.tile([128, 128], F32R)
            nc.vector.tensor_copy(ident[:], identF[:])
            ident_bf128 = const.tile([128, 128], BF16)
            nc.vector.tensor_copy(ident_bf128[:], identF[:])
            scrF = const.tile([128, 128], F32)
            nc.vector.memset(scrF[:], 0.0)
            zpad = const.tile([128, 8], BF16)
            nc.vector.tensor_copy(zpad[:], scrF[:, 0:8])
            nc.vector.memset(scrF[0:1, :], 1.0)
            ones = const.tile([1, 128], F32R)
            nc.vector.tensor_copy(ones[:], scrF[0:1, :])
            ident_bf = const.tile([32, 32], BF16)
            nc.vector.tensor_copy(ident_bf[:], identF[0:32, 0:32])
            nc.scalar.activation(scrF[96:97, 0:8], scrF[96:97, 0:8],
                                 AF.Sigmoid)
            ones_bf = const.tile([1, 128], BF16)
            nc.vector.tensor_copy(ones_bf[:], scrF[0:1, :])
            # blockmask[k, p] = 1 iff p // 32 == k  (K=4 stationary that maps
            # 4 bias rows onto the 4 32-partition blocks in one matmul)
            blockmask = const.tile([4, 128], BF16)
            nc.sync.dma_start(blockmask[:], blockmask_in)

            # phase-2 weight pool entered before phase-1 pools so the
            # pool stack pops cleanly (DMAs are issued mid-phase-1)
            rw = ctx.enter_context(tc.tile_pool(name="rw", bufs=1))

            # =========================================================
            # Phase 1: encoder (my BL batches) + Xdec GEMM (all rows)
            # =========================================================
            with ExitStack() as p1:
                wpool1 = p1.enter_context(tc.tile_pool(name="wpool1", bufs=1))
                gpool = p1.enter_context(tc.tile_pool(name="gpool", bufs=16))
                tpp = p1.enter_context(tc.tile_pool(name="tpp", bufs=2,
                                                    space="PSUM"))
                cps = p1.enter_context(tc.tile_pool(name="cps", bufs=3,
                                                    space="PSUM"))
                fps = p1.enter_context(tc.tile_pool(name="fps", bufs=3,
                                                    space="PSUM"))
                p1e = ExitStack()
                encp = p1e.enter_context(tc.tile_pool(name="encp", bufs=1))

                # ---- index DMAs FIRST so gathers start immediately ----
                idx_s_sb = encp.tile([128, (BL * S) // 128], dt.int32)
                nc.sync.dma_start(idx_s_sb[:], src_idx)
                idx_d_sb = wpool1.tile([128, NCH], dt.int32)
                nc.sync.dma_start(idx_d_sb[:], dec_idx)

                # ---- issue all enc-embedding gathers up front ----
                gts = []
                for b in range(BL):
                    for ch in range(4):
                        gt = gpool.tile([128, E], F32, tag="gath")
                        nc.gpsimd.indirect_dma_start(
                            out=gt[:], out_offset=None, in_=enc_emb,
                            in_offset=bass.IndirectOffsetOnAxis(
                                ap=idx_s_sb[:, 4 * b + ch:4 * b + ch + 1],
                                axis=0))
                        gts.append(gt)

                # ---- weight DMAs (behind the gathers in queue order) ----
                wconv_sb = {}
                for k in FS:
                    wk = encp.tile([128, k * 4 * 128], BF16,
                                   name=f"wconv{k}_sb")
                    nc.sync.dma_start(wk[:], wconv[k])
                    wconv_sb[k] = wk
                bconv_sb = encp.tile([128, 2 * len(FS)], F32)
                nc.sync.dma_start(bconv_sb[:], bconv)
                fc1T_sb = encp.tile([128, 6 * H], F32R)
                nc.sync.dma_start(fc1T_sb[:], fc1T)
                fc2T_sb = encp.tile([128, 4 * H], F32R)
                nc.sync.dma_start(fc2T_sb[:], fc2T)
                fc1b_sb = encp.tile([1, H], F32R)
                nc.sync.dma_start(fc1b_sb[:], fc1b)
                fc2b_sb = encp.tile([1, H], F32R)
                nc.sync.dma_start(fc2b_sb[:], fc2b)
                WeT_sb = encp.tile([128, 4 * G4], BF16)
                nc.sync.dma_start(WeT_sb[:], WeT)
                b1_sb = encp.tile([1, G4], BF16)
                nc.sync.dma_start(b1_sb[:], b1row)
                WdT_sb = wpool1.tile([128, 2 * G4], BF16)
                nc.sync.dma_start(WdT_sb[:], WdT)

                XPAD = BL * (S + 8)
                SEG = S + 8
                xT_sb = encp.tile([128, 2 * XPAD], BF16)        # [ec] blocks
                dembT_sb = wpool1.tile([128, 2 * RPAD], BF16)   # [ec] blocks

                def evict(dst, src, parity):
                    if parity % 2 == 0:
                        nc.vector.tensor_copy(dst, src)
                    else:
                        nc.scalar.copy(dst, src)

                pooled = encp.tile([128, 6 * BL], F32R)

                def conv_batch(b):
                    for ki, k in enumerate(FS):
                        for fc in range(2):
                            ps = cps.tile([128, 512], F32, tag="conv",
                                          space="PSUM")
                            first = True
                            for j in range(k):
                                for ec in range(2):
                                    lhs = wconv_sb[k][
                                        :, (j * 4 + ec * 2 + fc) * 128:
                                        (j * 4 + ec * 2 + fc) * 128 + 128]
                                    rhs = xT_sb[:, ec * XPAD + SEG * b + j:
                                                ec * XPAD + SEG * b + j + 512]
                                    nc.tensor.matmul(
                                        ps[:], lhs, rhs, start=first,
                                        stop=(j == k - 1 and ec == 1))
                                    first = False
                            kc = ki * 2 + fc
                            nc.vector.tensor_reduce(
                                pooled[:, BL * kc + b: BL * kc + b + 1],
                                ps[:, 0:S - k + 1], axis=AX.X, op=ALU.max)

                for b in range(BL):
                    for ch in range(4):
                        gt = gts[4 * b + ch]
                        for ec in range(2):
                            tp = tpp.tile([128, 128], F32, tag="tp",
                                          space="PSUM")
                            nc.tensor.transpose(
                                tp[:], gt[:, 128 * ec:128 * ec + 128],
                                ident[:].bitcast(F32))
                            evict(xT_sb[:, ec * XPAD + SEG * b + 128 * ch:
                                        ec * XPAD + SEG * b + 128 * ch + 128],
                                  tp[:], ch + ec)
                    for ec in range(2):
                        nc.vector.tensor_copy(
                            xT_sb[:, ec * XPAD + SEG * b + S:
                                  ec * XPAD + SEG * (b + 1)], zpad[:])
                    conv_batch(b)

                for ki in range(len(FS)):
                    for fc in range(2):
                        kc = ki * 2 + fc
                        nc.scalar.activation(
                            pooled[:, BL * kc: BL * kc + BL],
                            pooled[:, BL * kc: BL * kc + BL],
                            AF.Relu, bias=bconv_sb[:, fc * 3 + ki: fc * 3 + ki + 1])

                # ---- fc1 -> relu -> fc2 -> Xenc (into cc_in) ----
                ps1 = fps.tile([BL, H], F32, tag="f", space="PSUM")
                for kc in range(6):
                    nc.tensor.matmul(ps1[:], _r(pooled[:, BL * kc: BL * kc + BL]),
                                     _r(fc1T_sb[:, H * kc: H * kc + H]),
                                     start=(kc == 0), stop=False)
                nc.tensor.matmul(ps1[:], _r(ones[0:1, 0:BL]), _r(fc1b_sb[:]),
                                 start=False, stop=True)
                h1e = encp.tile([BL, H], F32)
                nc.scalar.activation(h1e[:], ps1[:], AF.Relu)

                h1eT = encp.tile([128, 4 * BL], F32R)
                for kc in range(4):
                    tp = tpp.tile([128, 128], F32, tag="tp", space="PSUM")
                    nc.tensor.transpose(tp[0:128, 0:BL],
                                        h1e[:, 128 * kc:128 * kc + 128],
                                        ident[0:BL, 0:BL].bitcast(F32))
                    nc.vector.tensor_copy(h1eT[:, BL * kc:BL * kc + BL],
                                          tp[0:128, 0:BL])

                ps2 = fps.tile([BL, H], F32, tag="f", space="PSUM")
                for kc in range(4):
                    nc.tensor.matmul(ps2[:], _r(h1eT[:, BL * kc:BL * kc + BL]),
                                     _r(fc2T_sb[:, H * kc:H * kc + H]),
                                     start=(kc == 0), stop=False)
                nc.tensor.matmul(ps2[:], _r(ones[0:1, 0:BL]), _r(fc2b_sb[:]),
                                 start=False, stop=True)
                enc_sb = encp.tile([BL, H], F32)
                nc.vector.tensor_copy(enc_sb[:], ps2[:])

                encT = encp.tile([128, 4 * BL], BF16)
                for kc in range(4):
                    tp = tpp.tile([128, 128], F32, tag="tp", space="PSUM")
                    nc.tensor.transpose(tp[0:128, 0:BL],
                                        enc_sb[:, 128 * kc:128 * kc + 128],
                                        ident[0:BL, 0:BL].bitcast(F32))
                    nc.vector.tensor_copy(encT[:, BL * kc:BL * kc + BL],
                                          tp[0:128, 0:BL])

                # xe = enc @ We^T + b1  (per-batch rows of the g1 preact)
                xe_sb = encp.tile([BL, G4], BF16)
                for n in range(4):
                    ps = fps.tile([BL, 512], F32, tag="f", space="PSUM")
                    for kc in range(4):
                        nc.tensor.matmul(
                            ps[:], encT[:, BL * kc:BL * kc + BL],
                            WeT_sb[:, kc * G4 + 512 * n:
                                   kc * G4 + 512 * n + 512],
                            start=(kc == 0), stop=False)
                    nc.tensor.matmul(ps[:], ones_bf[0:1, 0:BL],
                                     b1_sb[:, 512 * n:512 * n + 512],
                                     start=False, stop=True)
                    nc.vector.tensor_copy(xe_sb[:, 512 * n:512 * n + 512], ps[:])
                nc.sync.dma_start(cc_in[:], xe_sb[:])
                p1e.close()

                # ---- dec-token gathers BEFORE the collective so the Xdec
                # work overlaps the AllGather latency ----
                dgs = []
                for m in range(NM):
                    gt = gpool.tile([128, E], F32, tag="gath")
                    nc.gpsimd.indirect_dma_start(
                        out=gt[:], out_offset=None, in_=dec_emb,
                        in_offset=bass.IndirectOffsetOnAxis(
                            ap=idx_d_sb[:, m:m + 1], axis=0))
                    dgs.append(gt)

                nc.gpsimd.collective_compute(
                    "AllGather", ALU.bypass,
                    replica_groups=[list(range(NCORES))],
                    ins=[cc_in.opt()], outs=[cc_out.opt()])

                # phase-2 weight DMAs issued here so the 10MB loads overlap
                # the collective + Xdec GEMM instead of stalling decode start
                b2_sb = rw.tile([128, H], BF16)
                nc.sync.dma_start(b2_sb[:], b2pack)
                whh1_sb = rw.tile([128, 4 * G4], BF16)
                nc.sync.dma_start(whh1_sb[:], whh1T)
                wih2_sb = rw.tile([128, 4 * G4], BF16)
                nc.sync.dma_start(wih2_sb[:], wih2T)
                whh2_sb = rw.tile([128, 4 * G4], BF16)
                nc.sync.dma_start(whh2_sb[:], whh2T)
                ob8_sb = rw.tile([4, 2 * 512], BF16)
                nc.sync.dma_start(ob8_sb[:], ob8)

                for m in range(NM):
                    gt = dgs[m]
                    for ec in range(2):
                        tp = tpp.tile([128, 128], F32, tag="tp", space="PSUM")
                        nc.tensor.transpose(
                            tp[:], gt[:, 128 * ec:128 * ec + 128],
                            ident[:].bitcast(F32))
                        evict(dembT_sb[:, ec * RPAD + 128 * m:
                                       ec * RPAD + 128 * m + 128],
                              tp[:], m + ec)

                # ---- Xdec GEMM -> packed bf16 xih_dram ----
                xdpool = p1.enter_context(tc.tile_pool(name="xdpool", bufs=2))
                for m in range(NM):
                    tm = min(4, tt - 4 * m)
                    Mm = 32 * tm
                    xd_sb = xdpool.tile([128, G4], BF16, tag="xd_sb")
                    for n in range(4):
                        ps = fps.tile([128, 512], F32, tag="f", space="PSUM")
                        for ec in range(2):
                            nc.tensor.matmul(
                                ps[0:Mm, :],
                                dembT_sb[:, ec * RPAD + 128 * m:
                                         ec * RPAD + 128 * m + Mm],
                                WdT_sb[:, ec * G4 + 512 * n:
                                       ec * G4 + 512 * n + 512],
                                start=(ec == 0), stop=(ec == 1))
                        evict(xd_sb[0:Mm, 512 * n:512 * n + 512],
                              ps[0:Mm, :], m + n)
                    for tau in range(tm):
                        dst = xih_dram[4 * m + tau].rearrange(
                            "(j b) d -> b j d", j=4)
                        nc.sync.dma_start(dst,
                                          xd_sb[32 * tau:32 * tau + 32, :])

            # =========================================================
            # Phase 2: recurrence with packed gate PSUM, col-tiled GEMMs
            # gate-block order [i, f, o, g] on psum partitions [0:32,...]
            # =========================================================
            with ExitStack() as p2:
                vw = p2.enter_context(tc.tile_pool(name="vw", bufs=1))
                xp = p2.enter_context(tc.tile_pool(name="xp", bufs=3))
                # xih prefetches queued before the big owT load
                xih_t = xp.tile([128, H], BF16, tag="xih")
                nc.sync.dma_start(xih_t[:], xih_dram[0])
                xih_n = xp.tile([128, H], BF16, tag="xih")
                nc.sync.dma_start(xih_n[:], xih_dram[1])
                # Xenc contribution repacked to the (32*gate+b, d) gate layout
                xe_sb = vw.tile([128, H], BF16)
                nc.sync.dma_start(xe_sb[:],
                                  cc_out.rearrange("b (j d) -> j b d", j=4))
                owT_sb = vw.tile([128, 4 * VSP], BF16)
                nc.sync.dma_start(owT_sb[:], owT)

                rp = p2.enter_context(tc.tile_pool(name="rp", bufs=2))
                hp = p2.enter_context(tc.tile_pool(name="hp", bufs=2))
                rps = p2.enter_context(tc.tile_pool(name="rps", bufs=2,
                                                    space="PSUM"))
                tps = p2.enter_context(tc.tile_pool(name="tps", bufs=1,
                                                    space="PSUM"))
                vo = p2.enter_context(tc.tile_pool(name="vo", bufs=3))
                vps = p2.enter_context(tc.tile_pool(name="vps", bufs=3,
                                                    space="PSUM"))

                c1 = rp.tile([64, H], BF16, tag="c1")
                nc.vector.memset(c1[32:64, :], 0.0)
                c2 = rp.tile([64, H], BF16, tag="c2")
                nc.vector.memset(c2[32:64, :], 0.0)

                # --- LSTM cell stages (emitted interleaved across the two
                # layers so neither ACT nor DVE head-of-line blocks) ---
                def cell_act_pre(ps_g, tag):
                    sig = rp.tile([96, H], BF16, tag=f"sig{tag}")
                    nc.scalar.activation(sig[:], ps_g[0:96, :], AF.Sigmoid)
                    tg = rp.tile([32, H], BF16, tag=f"tg{tag}")
                    nc.scalar.activation(tg[:], ps_g[96:128, :], AF.Tanh)
                    return sig, tg

                def cell_dve_mid(sig, tg, c_prev, tag):
                    c_new = rp.tile([64, H], BF16, tag=f"c{tag}")
                    nc.vector.tensor_mul(c_new[32:64, :], sig[32:64, :],
                                         c_prev[32:64, :])
                    m1 = rp.tile([64, H], BF16, tag=f"m1{tag}")
                    nc.vector.tensor_mul(m1[32:64, :], sig[0:32, :], tg[:])
                    nc.vector.tensor_add(c_new[32:64, :], m1[32:64, :],
                                         c_new[32:64, :])
                    return c_new

                def cell_act_tanhc(c_new, tag):
                    th = rp.tile([96, H], BF16, tag=f"th{tag}")
                    nc.scalar.activation(th[64:96, :], c_new[32:64, :], AF.Tanh)
                    return th

                def cell_dve_h(sig, th, tag):
                    h = rp.tile([32, H], BF16, tag=f"h{tag}")
                    nc.vector.tensor_mul(h[:], sig[64:96, :], th[64:96, :])
                    return h

                def transpose_state(h, dst_sb):
                    """h (32, H) -> h^T kc-chunks packed (128, 4x32) in SBUF."""
                    tp = tps.tile([128, 128], BF16, tag="tps", space="PSUM")
                    for kc in range(4):
                        nc.tensor.transpose(tp[:, 32 * kc:32 * kc + 32],
                                            h[:, 128 * kc:128 * kc + 128],
                                            ident_bf[:])
                    nc.vector.tensor_copy(dst_sb[:], tp[:])

                def gemm_block(ps, stat, w_sb, final):
                    for kc in range(4):
                        for j in range(4):
                            nc.tensor.matmul(
                                ps[32 * j:32 * j + 32, :],
                                stat[:, 32 * kc:32 * kc + 32],
                                w_sb[:, kc * G4 + 512 * j:
                                     kc * G4 + 512 * j + 512],
                                start=False,
                                stop=(final and kc == 3 and j == 3),
                                skip_group_check=True,
                                tile_position=(0, 32 * j))

                def vocab_step(t, h2T):
                    """Full padded-vocab GEMM for decode step t: two 4-way
                    column-tiled (128, 512) psum groups, opened by a K=4
                    blockmask x out_b matmul (bank clear + bias fold)."""
                    for g in range(2):
                        ps = vps.tile([128, 512], F32, tag="vps", space="PSUM")
                        nc.tensor.matmul(
                            ps[:], blockmask[:],
                            ob8_sb[:, 512 * g:512 * g + 512],
                            start=True, stop=False, skip_group_check=True)
                        for kc in range(4):
                            for j in range(4):
                                nc.tensor.matmul(
                                    ps[32 * j:32 * j + 32, :],
                                    h2T[:, 32 * kc:32 * kc + 32],
                                    owT_sb[:, kc * VSP + 512 * (4 * g + j):
                                           kc * VSP + 512 * (4 * g + j) + 512],
                                    start=False,
                                    stop=(kc == 3 and j == 3),
                                    skip_group_check=True,
                                    tile_position=(0, 32 * j))
                        ob = vo.tile([128, 512], BF16, tag="ob")
                        # spread evictions across DVE and ACT so neither
                        # queue's cell chain is lengthened
                        if g == 0:
                            nc.vector.tensor_copy(ob[:], ps[:])
                        else:
                            nc.scalar.copy(ob[:], ps[:])
                        dst = out_dram[32 * t:32 * t + 32,
                                       2048 * g:2048 * g + 2048].rearrange(
                            "b (j d) -> j b d", j=4)
                        nc.sync.dma_start(dst, ob[:])

                def g1_open(xih, close=False):
                    ps = rps.tile([128, H], F32, tag="g1", space="PSUM")
                    nc.tensor.matmul(ps[:], ident_bf128[:], xih[:],
                                     start=True, stop=False,
                                     skip_group_check=True)
                    nc.tensor.matmul(ps[:], ident_bf128[:], xe_sb[:],
                                     start=False, stop=close,
                                     skip_group_check=True)
                    return ps

                # ---- software-pipelined decode loop ----
                # t=0: h1(-1) = 0, so the group closes after xih + xe
                ps_g1 = g1_open(xih_t, close=True)
                sig1, tg1 = cell_act_pre(ps_g1, "1")
                c1 = cell_dve_mid(sig1, tg1, c1, "1")
                th1 = cell_act_tanhc(c1, "1")
                h1 = cell_dve_h(sig1, th1, "1")

                # g2(0) init: b2 only (h2(-1) = 0)
                ps_g2 = rps.tile([128, H], F32, tag="g2", space="PSUM")
                nc.tensor.matmul(ps_g2[:], ident_bf128[:], b2_sb[:],
                                 start=True, stop=False, skip_group_check=True)

                h2T_prev = None
                for t in range(tt):
                    last = t + 1 >= tt
                    # PE: A (h1T transpose), B (wih2 -> finish ps_g2(t))
                    h1T = hp.tile([128, 128], BF16, tag="h1T")
                    transpose_state(h1, h1T)
                    gemm_block(ps_g2, h1T, wih2_sb, True)
                    # ACT: cell2 activations
                    sig2, tg2 = cell_act_pre(ps_g2, "2")
                    # PE: C (ps_g1(t+1) = xih + xe + whh1 @ h1T(t))
                    if not last:
                        ps_g1 = g1_open(xih_n)
                        gemm_block(ps_g1, h1T, whh1_sb, True)
                    # DVE: cell2 c-update
                    c2 = cell_dve_mid(sig2, tg2, c2, "2")
                    # ACT: cell1 activations (ps_g1 ready ~when tg2 ends)
                    if not last:
                        sig1, tg1 = cell_act_pre(ps_g1, "1")
                    th2 = cell_act_tanhc(c2, "2")
                    if not last:
                        c1 = cell_dve_mid(sig1, tg1, c1, "1")
                        th1 = cell_act_tanhc(c1, "1")
                    h2 = cell_dve_h(sig2, th2, "2")
                    if not last:
                        h1 = cell_dve_h(sig1, th1, "1")
                        if t + 2 < tt:
                            xih_n = xp.tile([128, H], BF16, tag="xih")
                            nc.sync.dma_start(xih_n[:], xih_dram[t + 2])
                    # PE: G' vocab(t-1) fills the cell windows
                    if t > 0:
                        vocab_step(t - 1, h2T_prev)
                    # PE: D (h2T transpose), E/F (ps_g2(t+1) = b2 + whh2)
                    h2T_cur = hp.tile([128, 128], BF16, tag="h2T")
                    transpose_state(h2, h2T_cur)
                    if not last:
                        ps_g2 = rps.tile([128, H], F32, tag="g2", space="PSUM")
                        nc.tensor.matmul(ps_g2[:], ident_bf128[:], b2_sb[:],
                                         start=True, stop=False,
                                         skip_group_check=True)
                        gemm_block(ps_g2, h2T_cur, whh2_sb, False)
                    h2T_prev = h2T_cur

                vocab_step(tt - 1, h2T_prev)

    nc.compile()
    return nc


# =====================================================================
# Host side
# =====================================================================

def _bf16(a):
    import ml_dtypes
    return np.ascontiguousarray(a.astype(ml_dtypes.bfloat16))


def _chunk(a):
    """(c*128, X) -> (128, c*X): partition-chunked layout for SBUF tiles."""
    c = a.shape[0] // 128
    return np.ascontiguousarray(
        a.reshape(c, 128, -1).transpose(1, 0, 2).reshape(128, -1))


def host_prep(inputs, tt=TT):
    """Build per-core input maps from the full problem inputs."""
    R = tt * B
    NM = math.ceil(R / 128)
    f32 = lambda a: np.ascontiguousarray(np.asarray(a), dtype=np.float32)
    # gate permutation [i, f, o, g]
    perm = np.concatenate([np.arange(0, H), np.arange(H, 2 * H),
                           np.arange(3 * H, 4 * H), np.arange(2 * H, 3 * H)])

    src = np.asarray(inputs["src"])
    trg = np.asarray(inputs["trg"])

    w_ih1 = f32(inputs["w_ih1"])[perm]
    b1 = (f32(inputs["b_ih1"]) + f32(inputs["b_hh1"]))[perm][None, :]
    b2 = (f32(inputs["b_ih2"]) + f32(inputs["b_hh2"]))[perm]
    b2pack = np.ascontiguousarray(
        np.broadcast_to(b2.reshape(4, 1, H), (4, 32, H)).reshape(128, H))

    shared = {
        "enc_emb": f32(inputs["enc_emb"]),
        "dec_emb": f32(inputs["dec_emb"]),
        "bconv": np.ascontiguousarray(
            np.stack([f32(inputs[f"conv_b{k}"]).reshape(2, 128)[fc]
                      for fc in range(2) for k in FS], axis=1)),
        "fc1T": _chunk(f32(inputs["fc1_w"]).T),
        "fc1b": f32(inputs["fc1_b"])[None, :],
        "fc2T": _chunk(f32(inputs["fc2_w"]).T),
        "fc2b": f32(inputs["fc2_b"])[None, :],
        "WdT": _bf16(_chunk(np.ascontiguousarray(w_ih1[:, :E].T))),
        "WeT": _bf16(_chunk(np.ascontiguousarray(w_ih1[:, E:].T))),
        "b1row": _bf16(b1),
        "b2pack": _bf16(b2pack),
        "whh1T": _bf16(_chunk(np.ascontiguousarray(f32(inputs["w_hh1"])[perm].T))),
        "wih2T": _bf16(_chunk(np.ascontiguousarray(f32(inputs["w_ih2"])[perm].T))),
        "whh2T": _bf16(_chunk(np.ascontiguousarray(f32(inputs["w_hh2"])[perm].T))),
    }
    for k in FS:
        A = f32(inputs[f"conv_w{k}"]).transpose(2, 1, 0)   # (k, E, F)
        A = A.reshape(k, 2, 128, 2, 128).transpose(0, 1, 3, 2, 4)
        shared[f"wconv{k}"] = _bf16(_chunk(A.reshape(k * 4 * 128, 128)))

    dtoks = trg[:, :tt].T.reshape(-1).astype(np.int32)
    dtoks = np.concatenate([dtoks, np.zeros(NM * 128 - R, np.int32)])
    dec_idx = np.ascontiguousarray(dtoks.reshape(NM, 128).T)

    owT_full = np.ascontiguousarray(f32(inputs["out_w"]).T)   # (H, V)
    ob_full = f32(inputs["out_b"])

    in_maps = []
    for c in range(NCORES):
        stoks = src[BL * c: BL * (c + 1)].reshape(-1).astype(np.int32)
        m = dict(shared)
        m["src_idx"] = np.ascontiguousarray(stoks.reshape(-1, 128).T)
        m["dec_idx"] = dec_idx
        ow = np.zeros((H, VSP), np.float32)
        ow[:, :VS] = owT_full[:, VS * c: VS * (c + 1)]
        m["owT"] = _bf16(_chunk(ow))
        ob = np.zeros(VSP, np.float32)
        ob[:VS] = ob_full[VS * c: VS * (c + 1)]
        m["ob8"] = _bf16(np.ascontiguousarray(
            ob.reshape(2, 4, 512).transpose(1, 0, 2).reshape(4, 1024)))
        bm = np.zeros((4, 128), np.float32)
        for k in range(4):
            bm[k, 32 * k:32 * k + 32] = 1.0
        m["blockmask"] = _bf16(bm)
        in_maps.append(m)
    return in_maps


def assemble(results, tt=TT):
    """Gather per-core logit shards -> full (B, T, V) output."""
    out = np.zeros((B, T, V), dtype=np.float32)
    for c, res in enumerate(results):
        sh = np.asarray(res["logits_sh"]).astype(np.float32)
        sh = sh.reshape(tt, B, VSP)[:, :, :VS]
        out[:, 1:1 + tt, VS * c: VS * (c + 1)] = sh.transpose(1, 0, 2)
    return out


_CACHE = {}


def kernel(**inputs):
    if "nc" not in _CACHE:
        _CACHE["nc"] = build()
    nc = _CACHE["nc"]
    from concourse.bass_utils import run_bass_kernel_spmd
    in_maps = host_prep(inputs)
    res = run_bass_kernel_spmd(nc, in_maps, core_ids=list(range(NCORES)))
    return assemble(res.results)


# revision 25
# speedup vs baseline: 1.0940x; 1.0878x over previous
"""CNNSummarizer (CNN encoder + 2-layer LSTM decoder + vocab projection) on 8 trn2 cores.

Sharding:
  - encoder: data-parallel over batch (4 batches per core); one AllGather of the
    per-batch encoder contribution to the LSTM-1 input preactivation (32KB).
  - LSTM recurrence: replicated on all 8 cores (small-collective latency makes
    per-step sharding a loss).
  - vocab projection (H -> V GEMM): column-sharded, 4000 vocab per core (padded
    to 4096 so every vocab tile is 512 wide).

Key structure (v2):
  - Phase 1: index DMAs + embedding gathers are issued before the bulk weight
    DMAs; the dec-token gathers are issued BEFORE the AllGather so the Xdec
    GEMM overlaps the collective.  The encoder contribution Xenc and the layer-1
    bias are folded into the per-step xih tensor during phase 1 (one stacked-
    identity matmul per (m, n) chunk), so the decode loop needs a single
    identity-inject per step for layer 1.
  - Phase 2: per-step vocab GEMM.  Each step's h2^T (128, 4x32 K-chunks) is the
    matmul stationary for an 8x column-tiled (tile_position) vocab GEMM covering
    all 4096 padded vocab columns of this core, emitted in the PE stream at the
    position that covers the LSTM cell latency of the NEXT step.  This keeps the
    tensor engine dense (HAM stays un-throttled) and removes the old 4-step
    lagged 128-row vocab units and the big h2T_all buffer.
  - Logits leave the device as bf16 (half the DMA + faster PSUM eviction);
    host upcasts.

Host-side work is limited to input marshalling: dtype casts of index tensors,
weight transposes/permutations, and the final gather/reshape of the output.
"""

import math
from contextlib import ExitStack

import numpy as np

import concourse.bacc as bacc
import concourse.bass as bass
import concourse.mybir as mybir
import concourse.tile as tile
from concourse.masks import make_identity

V, E, H, F = 32000, 256, 512, 256
FS = (3, 4, 5)
B, S, T = 32, 512, 64
NCORES = 8
BL = B // NCORES          # batches per core
VS = V // NCORES          # true vocab shard per core
VSP = 4096                # padded vocab shard (8 tiles of 512)
TT = T - 1                # decode steps actually computed
G4 = 4 * H                # 2048 gates

dt = mybir.dt
F32 = dt.float32
F32R = dt.float32r
BF16 = dt.bfloat16
AF = mybir.ActivationFunctionType
ALU = mybir.AluOpType
AX = mybir.AxisListType


def _r(ap):
    """View an fp32 AP as float32r for full-rate PE matmuls."""
    return ap.bitcast(F32R)


def build(tt=TT, trace_sim=False):
    """Build the per-core program. All 8 cores run the same NEFF; sharding comes
    from per-core input values."""
    R = tt * B                       # rows of the (t, b) decode matrix
    NM = math.ceil(R / 128)          # m-chunks of decode rows
    NCH = NM                         # dec-token gather chunks (128 tokens each)
    RPAD = NM * 128

    nc = bacc.Bacc("TRN2", target_bir_lowering=False, debug=False,
                   num_devices=NCORES)

    def inp(name, shape, dtype=F32):
        return nc.dram_tensor(name, list(shape), dtype, kind="ExternalInput").ap()

    src_idx = inp("src_idx", (128, (BL * S) // 128), dt.int32)
    dec_idx = inp("dec_idx", (128, NCH), dt.int32)
    enc_emb = inp("enc_emb", (V, E))
    dec_emb = inp("dec_emb", (V, E))
    wconv = {k: inp(f"wconv{k}", (128, k * 4 * 128), BF16) for k in FS}
    bconv = inp("bconv", (128, 2 * len(FS)))   # col = fc*3 + k_idx
    fc1T = inp("fc1T", (128, 6 * H), F32R)
    fc1b = inp("fc1b", (1, H), F32R)
    fc2T = inp("fc2T", (128, 4 * H), F32R)
    fc2b = inp("fc2b", (1, H), F32R)
    WdT = inp("WdT", (128, 2 * G4), BF16)
    WeT = inp("WeT", (128, 4 * G4), BF16)
    b1row = inp("b1row", (1, G4), BF16)
    b2pack = inp("b2pack", (128, H), BF16)
    whh1T = inp("whh1T", (128, 4 * G4), BF16)
    wih2T = inp("wih2T", (128, 4 * G4), BF16)
    whh2T = inp("whh2T", (128, 4 * G4), BF16)
    owT = inp("owT", (128, 4 * VSP), BF16)
    ob8 = inp("ob8", (4, 2 * 512), BF16)  # [k, 512g+d] = out_b[2048g+512k+d]
    blockmask_in = inp("blockmask", (4, 128), BF16)

    out_dram = nc.dram_tensor("logits_sh", [R, VSP], BF16,
                              kind="ExternalOutput").ap()

    with tile.TileContext(nc, trace_sim=trace_sim) as tc:
        with ExitStack() as ctx:
            dram = ctx.enter_context(tc.tile_pool(name="dram", bufs=1,
                                                  space="DRAM"))
            xih_dram = dram.tile([tt, 128, H], BF16)
            cc_in = dram.tile([BL, G4], BF16)
            cc_out = dram.tile([B, G4], BF16, addr_space="Shared")

            const = ctx.enter_context(tc.tile_pool(name="const", bufs=1))
            identF = const.tile([128, 128], F32)
            make_identity(nc, identF[:])
            ident = const.tile([128, 128], F32R)
            nc.vector.tensor_copy(ident[:], identF[:])
            ident_bf128 = const.tile([128, 128], BF16)
            nc.vector.tensor_copy(ident_bf128[:], identF[:])
            scrF = const.tile([128, 128], F32)
            nc.vector.memset(scrF[:], 0.0)
            zpad = const.tile([128, 8], BF16)
            nc.vector.tensor_copy(zpad[:], scrF[:, 0:8])
            nc.vector.memset(scrF[0:1, :], 1.0)
            ones = const.tile([1, 128], F32R)
            nc.vector.tensor_copy(ones[:], scrF[0:1, :])
            ident_bf = const.tile([32, 32], BF16)
            nc.vector.tensor_copy(ident_bf[:], identF[0:32, 0:32])
            nc.scalar.activation(scrF[96:97, 0:8], scrF[96:97, 0:8],
                                 AF.Sigmoid)
            ones_bf = const.tile([1, 128], BF16)
            nc.vector.tensor_copy(ones_bf[:], scrF[0:1, :])
            # blockmask[k, p] = 1 iff p // 32 == k  (K=4 stationary that maps
            # 4 bias rows onto the 4 32-partition blocks in one matmul)
            blockmask = const.tile([4, 128], BF16)
            nc.sync.dma_start(blockmask[:], blockmask_in)

            # phase-2 weight pool entered before phase-1 pools so the
            # pool stack pops cleanly (DMAs are issued mid-phase-1)
            rw = ctx.enter_context(tc.tile_pool(name="rw", bufs=1))

            # =========================================================
            # Phase 1: encoder (my BL batches) + Xdec GEMM (all rows)
            # =========================================================
            with ExitStack() as p1:
                wpool1 = p1.enter_context(tc.tile_pool(name="wpool1", bufs=1))
                gpool = p1.enter_context(tc.tile_pool(name="gpool", bufs=16))
                tpp = p1.enter_context(tc.tile_pool(name="tpp", bufs=2,
                                                    space="PSUM"))
                cps = p1.enter_context(tc.tile_pool(name="cps", bufs=3,
                                                    space="PSUM"))
                fps = p1.enter_context(tc.tile_pool(name="fps", bufs=3,
                                                    space="PSUM"))
                p1e = ExitStack()
                encp = p1e.enter_context(tc.tile_pool(name="encp", bufs=1))

                # ---- index DMAs FIRST so gathers start immediately ----
                idx_s_sb = encp.tile([128, (BL * S) // 128], dt.int32)
                nc.sync.dma_start(idx_s_sb[:], src_idx)
                idx_d_sb = wpool1.tile([128, NCH], dt.int32)
                nc.sync.dma_start(idx_d_sb[:], dec_idx)

                # ---- issue all enc-embedding gathers up front ----
                gts = []
                for b in range(BL):
                    for ch in range(4):
                        gt = gpool.tile([128, E], F32, tag="gath")
                        nc.gpsimd.indirect_dma_start(
                            out=gt[:], out_offset=None, in_=enc_emb,
                            in_offset=bass.IndirectOffsetOnAxis(
                                ap=idx_s_sb[:, 4 * b + ch:4 * b + ch + 1],
                                axis=0))
                        gts.append(gt)

                # ---- weight DMAs (behind the gathers in queue order) ----
                wconv_sb = {}
                for k in FS:
                    wk = encp.tile([128, k * 4 * 128], BF16,
                                   name=f"wconv{k}_sb")
                    nc.sync.dma_start(wk[:], wconv[k])
                    wconv_sb[k] = wk
                bconv_sb = encp.tile([128, 2 * len(FS)], F32)
                nc.sync.dma_start(bconv_sb[:], bconv)
                fc1T_sb = encp.tile([128, 6 * H], F32R)
                nc.sync.dma_start(fc1T_sb[:], fc1T)
                fc2T_sb = encp.tile([128, 4 * H], F32R)
                nc.sync.dma_start(fc2T_sb[:], fc2T)
                fc1b_sb = encp.tile([1, H], F32R)
                nc.sync.dma_start(fc1b_sb[:], fc1b)
                fc2b_sb = encp.tile([1, H], F32R)
                nc.sync.dma_start(fc2b_sb[:], fc2b)
                WeT_sb = encp.tile([128, 4 * G4], BF16)
                nc.sync.dma_start(WeT_sb[:], WeT)
                b1_sb = encp.tile([1, G4], BF16)
                nc.sync.dma_start(b1_sb[:], b1row)
                WdT_sb = wpool1.tile([128, 2 * G4], BF16)
                nc.sync.dma_start(WdT_sb[:], WdT)

                XPAD = BL * (S + 8)
                SEG = S + 8
                xT_sb = encp.tile([128, 2 * XPAD], BF16)        # [ec] blocks
                dembT_sb = wpool1.tile([128, 2 * RPAD], BF16)   # [ec] blocks

                def evict(dst, src, parity):
                    if parity % 2 == 0:
                        nc.vector.tensor_copy(dst, src)
                    else:
                        nc.scalar.copy(dst, src)

                pooled = encp.tile([128, 6 * BL], F32R)

                def conv_batch(b):
                    for ki, k in enumerate(FS):
                        for fc in range(2):
                            ps = cps.tile([128, 512], F32, tag="conv",
                                          space="PSUM")
                            first = True
                            for j in range(k):
                                for ec in range(2):
                                    lhs = wconv_sb[k][
                                        :, (j * 4 + ec * 2 + fc) * 128:
                                        (j * 4 + ec * 2 + fc) * 128 + 128]
                                    rhs = xT_sb[:, ec * XPAD + SEG * b + j:
                                                ec * XPAD + SEG * b + j + 512]
                                    nc.tensor.matmul(
                                        ps[:], lhs, rhs, start=first,
                                        stop=(j == k - 1 and ec == 1))
                                    first = False
                            kc = ki * 2 + fc
                            nc.vector.tensor_reduce(
                                pooled[:, BL * kc + b: BL * kc + b + 1],
                                ps[:, 0:S - k + 1], axis=AX.X, op=ALU.max)

                for b in range(BL):
                    for ch in range(4):
                        gt = gts[4 * b + ch]
                        for ec in range(2):
                            tp = tpp.tile([128, 128], F32, tag="tp",
                                          space="PSUM")
                            nc.tensor.transpose(
                                tp[:], gt[:, 128 * ec:128 * ec + 128],
                                ident[:].bitcast(F32))
                            evict(xT_sb[:, ec * XPAD + SEG * b + 128 * ch:
                                        ec * XPAD + SEG * b + 128 * ch + 128],
                                  tp[:], ch + ec)
                    for ec in range(2):
                        nc.vector.tensor_copy(
                            xT_sb[:, ec * XPAD + SEG * b + S:
                                  ec * XPAD + SEG * (b + 1)], zpad[:])
                    conv_batch(b)

                for ki in range(len(FS)):
                    for fc in range(2):
                        kc = ki * 2 + fc
                        nc.scalar.activation(
                            pooled[:, BL * kc: BL * kc + BL],
                            pooled[:, BL * kc: BL * kc + BL],
                            AF.Relu, bias=bconv_sb[:, fc * 3 + ki: fc * 3 + ki + 1])

                # ---- fc1 -> relu -> fc2 -> Xenc (into cc_in) ----
                ps1 = fps.tile([BL, H], F32, tag="f", space="PSUM")
                for kc in range(6):
                    nc.tensor.matmul(ps1[:], _r(pooled[:, BL * kc: BL * kc + BL]),
                                     _r(fc1T_sb[:, H * kc: H * kc + H]),
                                     start=(kc == 0), stop=False)
                nc.tensor.matmul(ps1[:], _r(ones[0:1, 0:BL]), _r(fc1b_sb[:]),
                                 start=False, stop=True)
                h1e = encp.tile([BL, H], F32)
                nc.scalar.activation(h1e[:], ps1[:], AF.Relu)

                h1eT = encp.tile([128, 4 * BL], F32R)
                for kc in range(4):
                    tp = tpp.tile([128, 128], F32, tag="tp", space="PSUM")
                    nc.tensor.transpose(tp[0:128, 0:BL],
                                        h1e[:, 128 * kc:128 * kc + 128],
                                        ident[0:BL, 0:BL].bitcast(F32))
                    nc.vector.tensor_copy(h1eT[:, BL * kc:BL * kc + BL],
                                          tp[0:128, 0:BL])

                ps2 = fps.tile([BL, H], F32, tag="f", space="PSUM")
                for kc in range(4):
                    nc.tensor.matmul(ps2[:], _r(h1eT[:, BL * kc:BL * kc + BL]),
                                     _r(fc2T_sb[:, H * kc:H * kc + H]),
                                     start=(kc == 0), stop=False)
                nc.tensor.matmul(ps2[:], _r(ones[0:1, 0:BL]), _r(fc2b_sb[:]),
                                 start=False, stop=True)
                enc_sb = encp.tile([BL, H], F32)
                nc.vector.tensor_copy(enc_sb[:], ps2[:])

                encT = encp.tile([128, 4 * BL], BF16)
                for kc in range(4):
                    tp = tpp.tile([128, 128], F32, tag="tp", space="PSUM")
                    nc.tensor.transpose(tp[0:128, 0:BL],
                                        enc_sb[:, 128 * kc:128 * kc + 128],
                                        ident[0:BL, 0:BL].bitcast(F32))
                    nc.vector.tensor_copy(encT[:, BL * kc:BL * kc + BL],
                                          tp[0:128, 0:BL])

                # xe = enc @ We^T + b1  (per-batch rows of the g1 preact)
                xe_sb = encp.tile([BL, G4], BF16)
                for n in range(4):
                    ps = fps.tile([BL, 512], F32, tag="f", space="PSUM")
                    for kc in range(4):
                        nc.tensor.matmul(
                            ps[:], encT[:, BL * kc:BL * kc + BL],
                            WeT_sb[:, kc * G4 + 512 * n:
                                   kc * G4 + 512 * n + 512],
                            start=(kc == 0), stop=False)
                    nc.tensor.matmul(ps[:], ones_bf[0:1, 0:BL],
                                     b1_sb[:, 512 * n:512 * n + 512],
                                     start=False, stop=True)
                    nc.vector.tensor_copy(xe_sb[:, 512 * n:512 * n + 512], ps[:])
                nc.sync.dma_start(cc_in[:], xe_sb[:])
                p1e.close()

                # ---- dec-token gathers BEFORE the collective so the Xdec
                # work overlaps the AllGather latency ----
                dgs = []
                for m in range(NM):
                    gt = gpool.tile([128, E], F32, tag="gath")
                    nc.gpsimd.indirect_dma_start(
                        out=gt[:], out_offset=None, in_=dec_emb,
                        in_offset=bass.IndirectOffsetOnAxis(
                            ap=idx_d_sb[:, m:m + 1], axis=0))
                    dgs.append(gt)

                nc.gpsimd.collective_compute(
                    "AllGather", ALU.bypass,
                    replica_groups=[list(range(NCORES))],
                    ins=[cc_in.opt()], outs=[cc_out.opt()])

                # phase-2 weight DMAs issued here so the 10MB loads overlap
                # the collective + Xdec GEMM instead of stalling decode start
                b2_sb = rw.tile([128, H], BF16)
                nc.sync.dma_start(b2_sb[:], b2pack)
                whh1_sb = rw.tile([128, 4 * G4], BF16)
                nc.sync.dma_start(whh1_sb[:], whh1T)
                wih2_sb = rw.tile([128, 4 * G4], BF16)
                nc.sync.dma_start(wih2_sb[:], wih2T)
                whh2_sb = rw.tile([128, 4 * G4], BF16)
                nc.sync.dma_start(whh2_sb[:], whh2T)
                ob8_sb = rw.tile([4, 2 * 512], BF16)
                nc.sync.dma_start(ob8_sb[:], ob8)

                for m in range(NM):
                    gt = dgs[m]
                    for ec in range(2):
                        tp = tpp.tile([128, 128], F32, tag="tp", space="PSUM")
                        nc.tensor.transpose(
                            tp[:], gt[:, 128 * ec:128 * ec + 128],
                            ident[:].bitcast(F32))
                        evict(dembT_sb[:, ec * RPAD + 128 * m:
                                       ec * RPAD + 128 * m + 128],
                              tp[:], m + ec)

                # ---- Xdec GEMM -> packed bf16 xih_dram ----
                xdpool = p1.enter_context(tc.tile_pool(name="xdpool", bufs=2))
                for m in range(NM):
                    tm = min(4, tt - 4 * m)
                    Mm = 32 * tm
                    xd_sb = xdpool.tile([128, G4], BF16, tag="xd_sb")
                    for n in range(4):
                        ps = fps.tile([128, 512], F32, tag="f", space="PSUM")
                        for ec in range(2):
                            nc.tensor.matmul(
                                ps[0:Mm, :],
                                dembT_sb[:, ec * RPAD + 128 * m:
                                         ec * RPAD + 128 * m + Mm],
                                WdT_sb[:, ec * G4 + 512 * n:
                                       ec * G4 + 512 * n + 512],
                                start=(ec == 0), stop=(ec == 1))
                        evict(xd_sb[0:Mm, 512 * n:512 * n + 512],
                              ps[0:Mm, :], m + n)
                    for tau in range(tm):
                        dst = xih_dram[4 * m + tau].rearrange(
                            "(j b) d -> b j d", j=4)
                        nc.sync.dma_start(dst,
                                          xd_sb[32 * tau:32 * tau + 32, :])

            # =========================================================
            # Phase 2: recurrence with packed gate PSUM, col-tiled GEMMs
            # gate-block order [i, f, o, g] on psum partitions [0:32,...]
            # =========================================================
            with ExitStack() as p2:
                vw = p2.enter_context(tc.tile_pool(name="vw", bufs=1))
                xp = p2.enter_context(tc.tile_pool(name="xp", bufs=3))
                # xih prefetches queued before the big owT load
                xih_t = xp.tile([128, H], BF16, tag="xih")
                nc.sync.dma_start(xih_t[:], xih_dram[0])
                xih_n = xp.tile([128, H], BF16, tag="xih")
                nc.sync.dma_start(xih_n[:], xih_dram[1])
                # Xenc contribution repacked to the (32*gate+b, d) gate layout
                xe_sb = vw.tile([128, H], BF16)
                nc.sync.dma_start(xe_sb[:],
                                  cc_out.rearrange("b (j d) -> j b d", j=4))
                owT_sb = vw.tile([128, 4 * VSP], BF16)
                nc.sync.dma_start(owT_sb[:], owT)

                rp = p2.enter_context(tc.tile_pool(name="rp", bufs=2))
                hp = p2.enter_context(tc.tile_pool(name="hp", bufs=2))
                rps = p2.enter_context(tc.tile_pool(name="rps", bufs=2,
                                                    space="PSUM"))
                tps = p2.enter_context(tc.tile_pool(name="tps", bufs=1,
                                                    space="PSUM"))
                vo = p2.enter_context(tc.tile_pool(name="vo", bufs=3))
                vps = p2.enter_context(tc.tile_pool(name="vps", bufs=2,
                                                    space="PSUM"))

                c1 = rp.tile([64, H], BF16, tag="c1")
                nc.vector.memset(c1[32:64, :], 0.0)
                c2 = rp.tile([64, H], BF16, tag="c2")
                nc.vector.memset(c2[32:64, :], 0.0)

                # --- LSTM cell stages (emitted interleaved across the two
                # layers so neither ACT nor DVE head-of-line blocks) ---
                def cell_act_pre(ps_g, tag):
                    sig = rp.tile([96, H], BF16, tag=f"sig{tag}")
                    nc.scalar.activation(sig[:], ps_g[0:96, :], AF.Sigmoid)
                    tg = rp.tile([32, H], BF16, tag=f"tg{tag}")
                    nc.scalar.activation(tg[:], ps_g[96:128, :], AF.Tanh)
                    return sig, tg

                def cell_dve_mid(sig, tg, c_prev, tag):
                    c_new = rp.tile([64, H], BF16, tag=f"c{tag}")
                    nc.vector.tensor_mul(c_new[32:64, :], sig[32:64, :],
                                         c_prev[32:64, :])
                    m1 = rp.tile([64, H], BF16, tag=f"m1{tag}")
                    nc.vector.tensor_mul(m1[32:64, :], sig[0:32, :], tg[:])
                    nc.vector.tensor_add(c_new[32:64, :], m1[32:64, :],
                                         c_new[32:64, :])
                    return c_new

                def cell_act_tanhc(c_new, tag):
                    th = rp.tile([96, H], BF16, tag=f"th{tag}")
                    nc.scalar.activation(th[64:96, :], c_new[32:64, :], AF.Tanh)
                    return th

                def cell_dve_h(sig, th, tag):
                    h = rp.tile([32, H], BF16, tag=f"h{tag}")
                    nc.vector.tensor_mul(h[:], sig[64:96, :], th[64:96, :])
                    return h

                def transpose_state(h, dst_sb):
                    """h (32, H) -> h^T kc-chunks packed (128, 4x32) in SBUF."""
                    tp = tps.tile([128, 128], BF16, tag="tps", space="PSUM")
                    for kc in range(4):
                        nc.tensor.transpose(tp[:, 32 * kc:32 * kc + 32],
                                            h[:, 128 * kc:128 * kc + 128],
                                            ident_bf[:])
                    nc.vector.tensor_copy(dst_sb[:], tp[:])

                def gemm_block(ps, stat, w_sb, final):
                    for kc in range(4):
                        for j in range(4):
                            nc.tensor.matmul(
                                ps[32 * j:32 * j + 32, :],
                                stat[:, 32 * kc:32 * kc + 32],
                                w_sb[:, kc * G4 + 512 * j:
                                     kc * G4 + 512 * j + 512],
                                start=False,
                                stop=(final and kc == 3 and j == 3),
                                skip_group_check=True,
                                tile_position=(0, 32 * j))

                def vocab_step(t, h2T):
                    """Full padded-vocab GEMM for decode step t: two 4-way
                    column-tiled (128, 512) psum groups, opened by a K=4
                    blockmask x out_b matmul (bank clear + bias fold)."""
                    for g in range(2):
                        ps = vps.tile([128, 512], F32, tag="vps", space="PSUM")
                        nc.tensor.matmul(
                            ps[:], blockmask[:],
                            ob8_sb[:, 512 * g:512 * g + 512],
                            start=True, stop=False, skip_group_check=True)
                        for kc in range(4):
                            for j in range(4):
                                nc.tensor.matmul(
                                    ps[32 * j:32 * j + 32, :],
                                    h2T[:, 32 * kc:32 * kc + 32],
                                    owT_sb[:, kc * VSP + 512 * (4 * g + j):
                                           kc * VSP + 512 * (4 * g + j) + 512],
                                    start=False,
                                    stop=(kc == 3 and j == 3),
                                    skip_group_check=True,
                                    tile_position=(0, 32 * j))
                        ob = vo.tile([128, 512], BF16, tag="ob")
                        # spread evictions across DVE and ACT so neither
                        # queue's cell chain is lengthened
                        if g == 0:
                            nc.vector.tensor_copy(ob[:], ps[:])
                        else:
                            nc.scalar.copy(ob[:], ps[:])
                        dst = out_dram[32 * t:32 * t + 32,
                                       2048 * g:2048 * g + 2048].rearrange(
                            "b (j d) -> j b d", j=4)
                        nc.sync.dma_start(dst, ob[:])

                def g1_open(xeih, close=False):
                    ps = rps.tile([128, H], F32, tag="g1", space="PSUM",
                                  bufs=3)
                    nc.tensor.matmul(ps[:], ident_bf128[:], xeih[:],
                                     start=True, stop=close,
                                     skip_group_check=True)
                    return ps

                # ---- software-pipelined decode loop ----
                xeih_t = xp.tile([128, H], BF16, tag="xeih")
                nc.vector.tensor_add(xeih_t[:], xih_t[:], xe_sb[:])
                xeih_n = xp.tile([128, H], BF16, tag="xeih")
                nc.vector.tensor_add(xeih_n[:], xih_n[:], xe_sb[:])
                # t=0: h1(-1) = 0, so the group closes after xih + xe
                ps_g1 = g1_open(xeih_t, close=True)
                sig1, tg1 = cell_act_pre(ps_g1, "1")
                c1 = cell_dve_mid(sig1, tg1, c1, "1")
                th1 = cell_act_tanhc(c1, "1")
                h1 = cell_dve_h(sig1, th1, "1")

                # g2(0) init: b2 only (h2(-1) = 0)
                ps_g2 = rps.tile([128, H], F32, tag="g2", space="PSUM")
                nc.tensor.matmul(ps_g2[:], ident_bf128[:], b2_sb[:],
                                 start=True, stop=False, skip_group_check=True)

                h2T_prev = None
                for t in range(tt):
                    last = t + 1 >= tt
                    # PE: A (h1T transpose), B (wih2 -> finish ps_g2(t))
                    h1T = hp.tile([128, 128], BF16, tag="h1T")
                    transpose_state(h1, h1T)
                    gemm_block(ps_g2, h1T, wih2_sb, True)
                    # ACT: cell2 activations
                    sig2, tg2 = cell_act_pre(ps_g2, "2")
                    # PE: C (ps_g1(t+1) = xih + xe + whh1 @ h1T(t))
                    if not last:
                        ps_g1 = g1_open(xeih_n)
                        gemm_block(ps_g1, h1T, whh1_sb, True)
                    # DVE: cell2 c-update
                    c2 = cell_dve_mid(sig2, tg2, c2, "2")
                    # ACT: cell1 activations (ps_g1 ready ~when tg2 ends)
                    if not last:
                        sig1, tg1 = cell_act_pre(ps_g1, "1")
                    th2 = cell_act_tanhc(c2, "2")
                    if not last:
                        c1 = cell_dve_mid(sig1, tg1, c1, "1")
                        th1 = cell_act_tanhc(c1, "1")
                    h2 = cell_dve_h(sig2, th2, "2")
                    if not last:
                        h1 = cell_dve_h(sig1, th1, "1")
                        if t + 2 < tt:
                            xih_n = xp.tile([128, H], BF16, tag="xih")
                            nc.sync.dma_start(xih_n[:], xih_dram[t + 2])
                            xeih_n = xp.tile([128, H], BF16, tag="xeih")
                            nc.vector.tensor_add(xeih_n[:], xih_n[:],
                                                 xe_sb[:])
                    # PE: G' vocab(t-1) fills the cell windows
                    if t > 0:
                        vocab_step(t - 1, h2T_prev)
                    # PE: D (h2T transpose), E/F (ps_g2(t+1) = b2 + whh2)
                    h2T_cur = hp.tile([128, 128], BF16, tag="h2T")
                    transpose_state(h2, h2T_cur)
                    if not last:
                        ps_g2 = rps.tile([128, H], F32, tag="g2", space="PSUM")
                        nc.tensor.matmul(ps_g2[:], ident_bf128[:], b2_sb[:],
                                         start=True, stop=False,
                                         skip_group_check=True)
                        gemm_block(ps_g2, h2T_cur, whh2_sb, False)
                    h2T_prev = h2T_cur

                vocab_step(tt - 1, h2T_prev)

    nc.compile()
    return nc


# =====================================================================
# Host side
# =====================================================================

def _bf16(a):
    import ml_dtypes
    return np.ascontiguousarray(a.astype(ml_dtypes.bfloat16))


def _chunk(a):
    """(c*128, X) -> (128, c*X): partition-chunked layout for SBUF tiles."""
    c = a.shape[0] // 128
    return np.ascontiguousarray(
        a.reshape(c, 128, -1).transpose(1, 0, 2).reshape(128, -1))


def host_prep(inputs, tt=TT):
    """Build per-core input maps from the full problem inputs."""
    R = tt * B
    NM = math.ceil(R / 128)
    f32 = lambda a: np.ascontiguousarray(np.asarray(a), dtype=np.float32)
    # gate permutation [i, f, o, g]
    perm = np.concatenate([np.arange(0, H), np.arange(H, 2 * H),
                           np.arange(3 * H, 4 * H), np.arange(2 * H, 3 * H)])

    src = np.asarray(inputs["src"])
    trg = np.asarray(inputs["trg"])

    w_ih1 = f32(inputs["w_ih1"])[perm]
    b1 = (f32(inputs["b_ih1"]) + f32(inputs["b_hh1"]))[perm][None, :]
    b2 = (f32(inputs["b_ih2"]) + f32(inputs["b_hh2"]))[perm]
    b2pack = np.ascontiguousarray(
        np.broadcast_to(b2.reshape(4, 1, H), (4, 32, H)).reshape(128, H))

    shared = {
        "enc_emb": f32(inputs["enc_emb"]),
        "dec_emb": f32(inputs["dec_emb"]),
        "bconv": np.ascontiguousarray(
            np.stack([f32(inputs[f"conv_b{k}"]).reshape(2, 128)[fc]
                      for fc in range(2) for k in FS], axis=1)),
        "fc1T": _chunk(f32(inputs["fc1_w"]).T),
        "fc1b": f32(inputs["fc1_b"])[None, :],
        "fc2T": _chunk(f32(inputs["fc2_w"]).T),
        "fc2b": f32(inputs["fc2_b"])[None, :],
        "WdT": _bf16(_chunk(np.ascontiguousarray(w_ih1[:, :E].T))),
        "WeT": _bf16(_chunk(np.ascontiguousarray(w_ih1[:, E:].T))),
        "b1row": _bf16(b1),
        "b2pack": _bf16(b2pack),
        "whh1T": _bf16(_chunk(np.ascontiguousarray(f32(inputs["w_hh1"])[perm].T))),
        "wih2T": _bf16(_chunk(np.ascontiguousarray(f32(inputs["w_ih2"])[perm].T))),
        "whh2T": _bf16(_chunk(np.ascontiguousarray(f32(inputs["w_hh2"])[perm].T))),
    }
    for k in FS:
        A = f32(inputs[f"conv_w{k}"]).transpose(2, 1, 0)   # (k, E, F)
        A = A.reshape(k, 2, 128, 2, 128).transpose(0, 1, 3, 2, 4)
        shared[f"wconv{k}"] = _bf16(_chunk(A.reshape(k * 4 * 128, 128)))

    dtoks = trg[:, :tt].T.reshape(-1).astype(np.int32)
    dtoks = np.concatenate([dtoks, np.zeros(NM * 128 - R, np.int32)])
    dec_idx = np.ascontiguousarray(dtoks.reshape(NM, 128).T)

    owT_full = np.ascontiguousarray(f32(inputs["out_w"]).T)   # (H, V)
    ob_full = f32(inputs["out_b"])

    in_maps = []
    for c in range(NCORES):
        stoks = src[BL * c: BL * (c + 1)].reshape(-1).astype(np.int32)
        m = dict(shared)
        m["src_idx"] = np.ascontiguousarray(stoks.reshape(-1, 128).T)
        m["dec_idx"] = dec_idx
        ow = np.zeros((H, VSP), np.float32)
        ow[:, :VS] = owT_full[:, VS * c: VS * (c + 1)]
        m["owT"] = _bf16(_chunk(ow))
        ob = np.zeros(VSP, np.float32)
        ob[:VS] = ob_full[VS * c: VS * (c + 1)]
        m["ob8"] = _bf16(np.ascontiguousarray(
            ob.reshape(2, 4, 512).transpose(1, 0, 2).reshape(4, 1024)))
        bm = np.zeros((4, 128), np.float32)
        for k in range(4):
            bm[k, 32 * k:32 * k + 32] = 1.0
        m["blockmask"] = _bf16(bm)
        in_maps.append(m)
    return in_maps


def assemble(results, tt=TT):
    """Gather per-core logit shards -> full (B, T, V) output."""
    out = np.zeros((B, T, V), dtype=np.float32)
    for c, res in enumerate(results):
        sh = np.asarray(res["logits_sh"]).astype(np.float32)
        sh = sh.reshape(tt, B, VSP)[:, :, :VS]
        out[:, 1:1 + tt, VS * c: VS * (c + 1)] = sh.transpose(1, 0, 2)
    return out


_CACHE = {}


def kernel(**inputs):
    if "nc" not in _CACHE:
        _CACHE["nc"] = build()
    nc = _CACHE["nc"]
    from concourse.bass_utils import run_bass_kernel_spmd
    in_maps = host_prep(inputs)
    res = run_bass_kernel_spmd(nc, in_maps, core_ids=list(range(NCORES)))
    return assemble(res.results)
